# revision 11
# baseline (speedup 1.0000x reference)
"""Cross-attention kernel for TRN2, 8 NeuronCores.

Sharding: core c -> (batch b = c//2, head-group g = c%2).  Each head-group is
8 heads = 512 of the 1024 d_model channels.  Within a core everything runs in
one fused software pipeline over 4 head-pair stages (o = 0..3):

  QT = wq_g.T @ q.T  (scale folded)       [512, 512]   (s, lq)
  KT = wk_g.T @ kv.T                      [512, 2048]  (s, lkv)
  V  = kv @ wv_g.T                        [2048, 512]  (lkv, s)   + ones col
  phase1(o), t = 0..15:
               ST = Kh.T-contract @ QT    [128, 512]   (lkv-tile, lq)
               P[t] = exp(ST)          -> bf16 SBUF [128, 16, 512] per head
  phase2(o), unit (hp, lt):  16 consecutive matmuls in ONE psum bank
               ctx[lq, 65] += P[t]_lt.T @ [Vh | 1]     (F=65 transposed form;
                                                        col 64 = softmax denom)
               C = ctx[:, 0:64] * recip(ctx[:, 64])  (DVE per-partition scalar)
  transpose C -> cT[s, lq]  (PE transpose, identity trick)
  out = cT.T @ wo_g.T                     [512, 1024]
Host sums the two head-group partials per batch and adds bo + bv@Wo.T
(the V bias commutes through softmax-weighted averaging: sum(a)=1, so
ctx = sum(a v) + bv and the bv term is a constant row added on host; the
K bias drops entirely -- a per-row constant shift of the scores cancels
in softmax).

All operands are bf16 (1 cyc/row on PE at any free size; halves DMA), psum
accumulation f32.  phase2(o-1) and the projection matmuls of stage o+1 are
hand-interleaved into phase1(o)'s t-loop so the Act engine's exp stream (the
second largest engine load) fully overlaps PE work.  A psum accumulation
group owns its whole 2KB bank (start zeroes the full zero-region), hence the
consecutive-16 structure of phase2 rather than round-robin accumulation.

Timing model notes (TimelineSim is the metric): the PE clock ramp anchors at
the FIRST matmul and never resets on the gap sizes this kernel produces, so
three dummy matmuls right after the preamble are enough to have the ramp done
(3us) before the first real projection; DMA transfers serialize on the single
DMA_ENGINES device, so the input DMA order below is the exact just-in-time
consumption order of the stage-0 pipeline; the kernel tail is one
copy+DMA chain, minimized by making the final out-projection accumulation
group a [128,128] column slice whose DMA is the only one in flight.
"""

import sys
if "/opt/trn_rl_repo" not in sys.path:
    sys.path.insert(0, "/opt/trn_rl_repo")

import numpy as np
import ml_dtypes

import concourse.bass as bass
import concourse.mybir as mybir
import concourse.tile as tile
from concourse.bass_utils import run_bass_kernel_spmd

f32 = mybir.dt.float32
bf16 = mybir.dt.bfloat16
EXP = mybir.ActivationFunctionType.Exp
IDENT = mybir.ActivationFunctionType.Identity

D = 1024        # d_model
S = 512         # per-core channel shard (8 heads x 64)
LQ = 512
LKV = 2048
CO = D // 128   # 8 contraction chunks
SO = S // 128   # 4 shard s-tiles (head pairs)
NT = LKV // 128  # 16 lkv tiles


def _split_multi_waits(nc, max_waits=1):
    """This container's walrus allows only `max_waits` sync-wait commands per
    instruction; hoist the excess into standalone EventSemaphore insts."""
    ev_id = 0
    for f in nc.m.functions:
        for bb in f.blocks:
            new = []
            changed = False
            for inst in bb.instructions:
                si = inst.sync_info
                if si is not None and si.on_wait and len(si.on_wait) > max_waits:
                    waits = list(si.on_wait)
                    for sw in waits[:-max_waits]:
                        ev = mybir.InstEventSemaphore(
                            name=f"EVSPLIT-{ev_id}", engine=inst.engine,
                            sync_info=mybir.SyncInfo(on_wait=[sw], on_update=[]))
                        ev_id += 1
                        nc.register_instruction(ev, overwrite=True)
                        new.append(ev)
                    inst.sync_info = mybir.SyncInfo(
                        on_wait=waits[-max_waits:], on_update=list(si.on_update))
                    changed = True
                new.append(inst)
            if changed:
                bb.instructions = new
    return nc


def _build():
    nc = bass.Bass(trn_type="TRN2")

    # DRAM I/O (pre-laid-out [128, outer, free] on host, bf16)
    qT = nc.dram_tensor("qT", [128, CO, LQ], bf16, kind="ExternalInput")
    kvT = nc.dram_tensor("kvT", [128, CO, LKV], bf16, kind="ExternalInput")
    # o=0 slices of Wq|Wk packed contiguously: one full-rate DMA on the
    # critical path to the first score matmul (256-col slices of wqT/wkT
    # alone pay the sub-512B DMA penalty)
    wqk0 = nc.dram_tensor("wqk0", [128, CO, 256], bf16, kind="ExternalInput")
    # o=1..3 slices of Wq|Wk packed: cols 0:384 = wq[:, :, 128:512],
    # cols 384:768 = wk[:, :, 128:512]
    wqkr = nc.dram_tensor("wqkr", [128, CO, 768], bf16, kind="ExternalInput")
    wvT = nc.dram_tensor("wvT", [128, CO, S], bf16, kind="ExternalInput")
    woT = nc.dram_tensor("woT", [128, SO, D], bf16, kind="ExternalInput")
    bq = nc.dram_tensor("bq", [128, SO], f32, kind="ExternalInput")
    ident = nc.dram_tensor("ident", [128, 128], bf16, kind="ExternalInput")
    out = nc.dram_tensor("out", [SO, 128, D], bf16, kind="ExternalOutput")

    with tile.TileContext(nc) as tc:
        with tc.tile_pool(name="wgt", bufs=1) as wgt, \
             tc.tile_pool(name="pt", bufs=2) as ptp, \
             tc.tile_pool(name="stg", bufs=4) as stg, \
             tc.tile_pool(name="ost", bufs=3) as ost, \
             tc.tile_pool(name="ps", bufs=1, space="PSUM") as ps:

            # ---- resident SBUF ----
            kv_sb = wgt.tile([128, CO, LKV], bf16, name="kv_sb")
            wqkr_sb = wgt.tile([128, CO, 768], bf16, name="wqkr_sb")
            wv_sb = wgt.tile([128, CO, S], bf16, name="wv_sb")
            wo_sb = wgt.tile([128, SO, D], bf16, name="wo_sb")
            qT_sb = wgt.tile([128, CO, LQ], bf16, name="qT_sb")
            QT_sb = wgt.tile([128, SO, LQ], bf16, name="QT_sb")
            KT_sb = wgt.tile([128, SO, LKV], bf16, name="KT_sb")
            # V per head with a ones column: attn@V (transposed form) then
            # also yields the softmax denominator in output col 64.
            Vp_sb = wgt.tile([128, NT, 8, 65], bf16, name="Vp_sb")
            cT_sb = wgt.tile([128, SO, LQ], bf16, name="cT_sb")
            bq_sb = wgt.tile([128, SO], f32, name="bq_sb")
            ident_sb = wgt.tile([128, 128], bf16, name="ident_sb")
            wqk0_sb = wgt.tile([128, CO, 256], bf16, name="wqk0_sb")

            # ---- PE clock-ramp anchor: the ramp timer starts at the first
            # matmul and survives the idle gaps this kernel produces, so a
            # few cheap dummies right after the preamble put the engine at
            # full clock (3us later) before the first real projection.
            dm_sb = wgt.tile([128, 512], bf16, name="dm_sb")
            nc.vector.memset(dm_sb, 0.0)
            for i in range(8):
                dps = ps.tile([128, 512], f32, name=f"dps{i}", tag="proj",
                              bufs=2)
                nc.tensor.matmul(dps, dm_sb[:, 0:128], dm_sb,
                                 start=True, stop=True)

            # ---- DMA order = just-in-time consumption order (transfers
            # serialize on the DMA_ENGINES device).  Stage 0 needs the o=0
            # weight slices, q, and the kv chunks in score order; weights for
            # later stages and the out-projection arrive behind them.
            nc.sync.dma_start(wqk0_sb, wqk0[:])
            nc.sync.dma_start(qT_sb[:, 0:4, :], qT[:, 0:4, :])
            nc.sync.dma_start(qT_sb[:, 4:8, :], qT[:, 4:8, :])
            nc.sync.dma_start(kv_sb[:, 0:4, 0:512], kvT[:, 0:4, 0:512])
            nc.sync.dma_start(kv_sb[:, 4:8, 0:512], kvT[:, 4:8, 0:512])
            nc.sync.dma_start(bq_sb, bq[:])
            for hh in range(2, 8):
                nc.sync.dma_start(kv_sb[:, 4 * (hh % 2):4 * (hh % 2) + 4,
                                        (hh // 2) * 512:(hh // 2 + 1) * 512],
                                  kvT[:, 4 * (hh % 2):4 * (hh % 2) + 4,
                                      (hh // 2) * 512:(hh // 2 + 1) * 512])
            nc.sync.dma_start(wv_sb[:, :, 0:256], wvT[:, :, 0:256])
            nc.sync.dma_start(wv_sb[:, :, 256:512], wvT[:, :, 256:512])
            nc.sync.dma_start(ident_sb, ident[:])
            nc.sync.dma_start(wqkr_sb, wqkr[:])
            nc.sync.dma_start(wo_sb, woT[:])

            nc.vector.memset(Vp_sb[:, :, :, 64:65], 1.0)

            # ---- emission helpers (each emits PE matmuls + its drain) ----
            kps_open = {}   # (o, ch) -> open psum accumulation tile

            def kproj_half(o, ch, half):
                """Half of a K-projection accumulation group: c 0..3 (opens
                the psum group) or c 4..7 (closes it + drains to KT).  Stage
                0's kv chunks land one per ~1.5us, so splitting emission at
                the chunk boundary keeps the in-order PE from stalling a
                whole group's worth of score tiles behind one late chunk.
                NOTE: no other proj-tag tile may be allocated while a group
                is open (the 2-buffer rotation would land on the open bank).
                """
                if half == 0:
                    kps_open[(o, ch)] = ps.tile(
                        [128, 512], f32, name=f"kps{o}_{ch}", tag="proj",
                        bufs=2)
                kps = kps_open[(o, ch)]
                sl = slice(ch * 512, (ch + 1) * 512)
                for c in range(4 * half, 4 * half + 4):
                    w = (wqk0_sb[:, c, 128:256] if o == 0 else
                         wqkr_sb[:, c, 384 + (o - 1) * 128:384 + o * 128])
                    nc.tensor.matmul(kps, w, kv_sb[:, c, sl],
                                     start=(c == 0), stop=(c == CO - 1))
                if half == 1:
                    del kps_open[(o, ch)]
                    nc.vector.tensor_copy(KT_sb[:, o, sl], kps)

            def kproj(o, ch):
                kproj_half(o, ch, 0)
                kproj_half(o, ch, 1)

            def qproj(o):
                qps = ps.tile([128, 512], f32, name=f"qps{o}", tag="proj",
                              bufs=2)
                for c in range(CO):
                    w = (wqk0_sb[:, c, 0:128] if o == 0 else
                         wqkr_sb[:, c, (o - 1) * 128:o * 128])
                    nc.tensor.matmul(qps, w, qT_sb[:, c, :],
                                     start=(c == 0), stop=(c == CO - 1))
                nc.vector.tensor_scalar_add(QT_sb[:, o, :], qps,
                                            bq_sb[:, o:o + 1])

            def vproj(o, t):
                vps = ps.tile([128, 128], f32, name=f"vps{o}_{t}", tag="proj",
                              bufs=2)
                tsl = slice(t * 128, (t + 1) * 128)
                osl = slice(o * 128, (o + 1) * 128)
                for c in range(CO):
                    nc.tensor.matmul(vps, kv_sb[:, c, tsl], wv_sb[:, c, osl],
                                     start=(c == 0), stop=(c == CO - 1))
                nc.vector.tensor_copy(
                    Vp_sb[:, t, 2 * o:2 * o + 2, 0:64],
                    vps.rearrange("p (h d) -> p h d", h=2))

            # ---- lead-in: stage-0 prerequisites ----
            qproj(0)
            kproj(0, 0)

            # Per-stage fill schedules: iteration t -> thunks.  Placement
            # matches DMA arrival order (PE is in-order, so emitting a matmul
            # whose DMA lands late would stall everything behind it).
            def mk_sched(o):
                s = {t: [] for t in range(NT)}
                if o == 0:
                    # kv chunks land one per ~1.5us; kproj(0,ch) feeds the
                    # scores at t=4ch.  Emit each kproj as two chunk-halves
                    # at consecutive t so the PE stall per late chunk is
                    # spread across score tiles.  wv lands after kv7, so the
                    # V projections start one tile later than the kproj
                    # halves they share proj-tag banks with.
                    s[0].append(lambda: kproj_half(0, 1, 0))
                    s[1].append(lambda: kproj_half(0, 1, 1))
                    s[4].append(lambda: kproj_half(0, 2, 0))
                    s[5].append(lambda: kproj_half(0, 2, 1))
                    s[8].append(lambda: kproj_half(0, 3, 0))
                    s[9].append(lambda: kproj_half(0, 3, 1))
                    nv = 0
                    for t in range(9, NT):
                        take = 2 if t < 14 else 3
                        for _ in range(take):
                            if nv < NT:
                                s[t].append(lambda v=nv: vproj(0, v))
                                nv += 1
                else:
                    # own K chunks 1..3 first (ch0/qproj ran at the tail of
                    # the previous stage), V tiles just-in-time for phase2.
                    for ch in range(1, 4):
                        s[ch - 1].append(lambda ch=ch: kproj(o, ch))
                    for t in range(NT):
                        s[t].append(lambda t=t: vproj(o, t))
                if o < 3:
                    # next stage's first K chunk before its Q projection:
                    # the next stage's first score tile is gated by the
                    # KT-copy drain, so give it the extra headroom
                    s[NT - 2].append(lambda: kproj(o + 1, 0))
                    s[NT - 1].append(lambda: qproj(o + 1))
                return s

            def phase2_unit(o, pt, hp, lt, c_sb):
                """ctx unit (head hp of pair o, lq tile lt): 16 consecutive
                matmuls in one psum bank, then normalize straight from psum.
                (An accumulation group owns its whole 2KB zero-region, so the
                16 steps must be consecutive in one dedicated bank.)
                Pair 3 runs at the kernel tail where Act is idle, so its
                normalize goes to the scalar engine instead of DVE, and its
                units alternate over the then-idle proj banks as well to keep
                4 accumulations in flight instead of 2."""
                if o == SO - 1:
                    # score banks are idle after stage 3's last exp: rotate
                    # over ctx/st/proj so three accumulations stay in flight
                    tag = ("ctx", "st", "proj")[(hp * SO + lt) % 3]
                else:
                    tag = "ctx"
                ctx = ps.tile([128, 65], f32, name=f"ctx{o}_{hp}_{lt}",
                              tag=tag, bufs=2)
                base = hp * 512 + lt * 128
                for t in range(NT):
                    nc.tensor.matmul(
                        ctx, pt[:, t, base:base + 128],
                        Vp_sb[:, t, 2 * o + hp, :],
                        start=(t == 0), stop=(t == NT - 1))
                rc = stg.tile([128, 1], f32, name=f"rc{o}_{hp}_{lt}", tag="rc",
                              bufs=4)
                nc.vector.reciprocal(rc, ctx[:, 64:65])
                if o == SO - 1:
                    nc.scalar.activation(c_sb[:, hp, lt, :], ctx[:, 0:64],
                                         IDENT, scale=rc)
                else:
                    nc.vector.tensor_scalar_mul(
                        c_sb[:, hp, lt, :], ctx[:, 0:64], rc)

            def transpose_pair(o, hp, c_sb):
                trp = ps.tile([128, SO, 128], bf16, name=f"trp{o}_{hp}",
                              tag="proj", bufs=2)
                for lt in range(SO):
                    nc.tensor.transpose(trp[0:64, lt, :],
                                        c_sb[:, hp, lt, :], ident_sb)
                nc.vector.tensor_copy(
                    cT_sb[hp * 64:(hp + 1) * 64, o, :],
                    trp[0:64, :, :].rearrange("p a b -> p (a b)"))

            def phase2_steps(o, pt):
                """Thunks: 8 ctx units + 2 transposes for pair-stage o."""
                c_sb = stg.tile([128, 2, SO, 64], bf16, name=f"c{o}", tag="c",
                                bufs=2)
                for hp in range(2):
                    for lt in range(SO):
                        yield lambda hp=hp, lt=lt: phase2_unit(
                            o, pt, hp, lt, c_sb)
                    yield lambda hp=hp: transpose_pair(o, hp, c_sb)

            # ---- 4 head-pair stages ----
            prev_p2 = None   # phase2 step iterator of the previous stage
            for o in range(SO):
                sched = mk_sched(o)
                pt = ptp.tile([128, NT, 1024], bf16, name=f"pt{o}",
                              tag="pt", bufs=2)
                for t in range(NT):
                    # fused score tile: head 2o in bank cols 0:512, head
                    # 2o+1 in 512:1024 (each matmul stays within one bank)
                    st2 = ps.tile([128, 1024], f32, name=f"st{o}_{t}",
                                  tag="st", bufs=2)
                    tsl = slice(t * 128, (t + 1) * 128)
                    nc.tensor.matmul(st2[:, 0:512], KT_sb[0:64, o, tsl],
                                     QT_sb[0:64, o, :], start=True, stop=True)
                    nc.tensor.matmul(st2[:, 512:1024], KT_sb[64:128, o, tsl],
                                     QT_sb[64:128, o, :], start=True, stop=True)
                    nc.scalar.activation(pt[:, t, :], st2, EXP)
                    # one phase2 step of the previous stage every other t
                    if t % 2 == 1 and prev_p2 is not None:
                        step = next(prev_p2, None)
                        if step is not None:
                            step()
                        if t == NT - 1:  # 10 steps total, drain leftovers
                            for step in prev_p2:
                                step()
                    for thunk in sched[t]:
                        thunk()
                prev_p2 = phase2_steps(o, pt)
            for step in prev_p2:
                step()

            # ---- out projection: out[lq, d] += cT[:, o, lq-sl].T @ wo ----
            # The kernel end is gated by the LAST out-DMA chain: sem(~200) +
            # copy + HWDGE(625, single slot) + DGE(650) + transfer + 900.
            # Any other DMA issued within ~1.3us before it queues ahead of
            # it on HWDGE and becomes the laggard instead.  So the final lq
            # tile's last 128 columns are produced by TWO tiny [64]-column
            # groups (no DMA of their own) merged into ONE short final DMA,
            # and every other group's DMA is issued >= 2 group-slots
            # (~1.8us) before the end.
            ogroups = [
                (3, 0, 512, "act", "piece"),
                (0, 0, 512, "act", None), (0, 512, 1024, "dve", "full"),
                (1, 0, 512, "act", None), (1, 512, 1024, "dve", "full"),
                (2, 0, 512, "act", None), (2, 512, 1024, "dve", "full"),
            ] + [
                # lt3's high half as eight DMA-less [64]-col groups: ~0.9us
                # of PE work between the last big out-DMA (lt2) and kernel
                # end, so only the single merged DMA below is in flight at
                # the finish line.
                (3, 512 + 64 * i, 576 + 64 * i, ("act", "dve")[i % 2],
                 "tail" if i == 7 else None)
                for i in range(8)
            ]
            ot_tiles = {}
            for lt in range(SO):
                ot_tiles[lt] = ost.tile([128, D], bf16, name=f"ot{lt}",
                                        tag=f"ot{lt}")
            for gi, (lt, c0, c1, eng, dma) in enumerate(ogroups):
                lsl = slice(lt * 128, (lt + 1) * 128)
                ops = ps.tile([128, c1 - c0], f32, name=f"ops{gi}",
                              tag=("proj", "st")[gi % 2], bufs=2)
                for o in range(SO):
                    nc.tensor.matmul(ops, cT_sb[:, o, lsl],
                                     wo_sb[:, o, c0:c1],
                                     start=(o == 0), stop=(o == SO - 1))
                ot = ot_tiles[lt]
                if eng == "act":
                    nc.scalar.activation(ot[:, c0:c1], ops, IDENT)
                else:
                    nc.vector.tensor_copy(ot[:, c0:c1], ops)
                if dma == "piece":
                    nc.sync.dma_start(out[lt, :, c0:c1], ot[:, c0:c1])
                elif dma == "full":
                    nc.sync.dma_start(out[lt, :, :], ot)
                elif dma == "tail":
                    # covers all eight [64]-col groups; waits all copies
                    nc.sync.dma_start(out[lt, :, 512:1024], ot[:, 512:1024])

    return _split_multi_waits(nc)


_NC = None


def _get_nc():
    global _NC
    if _NC is None:
        _NC = _build()
    return _NC


def _shard(q, kv, Wq, bq, Wk, bk, Wv, bv, Wo, bo):
    b16 = ml_dtypes.bfloat16

    def lay(a2d, co):  # [co*128, F] -> [128, co, F]
        F = a2d.shape[1]
        return np.ascontiguousarray(
            a2d.reshape(co, 128, F).transpose(1, 0, 2)).astype(b16)

    idn = np.eye(128, dtype=b16)
    in_maps = []
    for core in range(8):
        b, g = core // 2, core % 2
        sl = slice(g * S, (g + 1) * S)
        wq_l = lay(np.ascontiguousarray((Wq[sl] * 0.125).T), CO)
        wk_l = lay(np.ascontiguousarray(Wk[sl].T), CO)
        m = {
            "wqk0": np.ascontiguousarray(
                np.concatenate([wq_l[:, :, 0:128], wk_l[:, :, 0:128]],
                               axis=2)),
            "wqkr": np.ascontiguousarray(
                np.concatenate([wq_l[:, :, 128:512], wk_l[:, :, 128:512]],
                               axis=2)),
            "qT": lay(np.ascontiguousarray(q[b].T), CO),
            "kvT": lay(np.ascontiguousarray(kv[b].T), CO),
            "wvT": lay(np.ascontiguousarray(Wv[sl].T), CO),
            "woT": lay(np.ascontiguousarray(Wo[:, sl].T), SO),
            "bq": np.ascontiguousarray(
                (bq[sl] * 0.125).reshape(SO, 128).T).astype(np.float32),
            "ident": idn,
        }
        in_maps.append(m)
    return in_maps


def _run(in_maps, trace=False):
    res = run_bass_kernel_spmd(_get_nc(), in_maps, core_ids=list(range(8)),
                               trace=trace)
    return res


def kernel(q, kv, Wq, bq, Wk, bk, Wv, bv, Wo, bo, _trace=False):
    q, kv = np.asarray(q, np.float32), np.asarray(kv, np.float32)
    Wq, Wk = np.asarray(Wq, np.float32), np.asarray(Wk, np.float32)
    Wv, Wo = np.asarray(Wv, np.float32), np.asarray(Wo, np.float32)
    bq, bk = np.asarray(bq, np.float32), np.asarray(bk, np.float32)
    bv, bo = np.asarray(bv, np.float32), np.asarray(bo, np.float32)

    in_maps = _shard(q, kv, Wq, bq, Wk, bk, Wv, bv, Wo, bo)
    res = _run(in_maps, trace=_trace)
    B = q.shape[0]
    # bv commutes through the softmax average; bk cancels in softmax.
    const_row = bv @ Wo.T + bo
    outp = np.empty((B, LQ, D), np.float32)
    for b in range(B):
        p0 = np.asarray(res.results[2 * b]["out"],
                        np.float32).reshape(LQ, D)
        p1 = np.asarray(res.results[2 * b + 1]["out"],
                        np.float32).reshape(LQ, D)
        outp[b] = p0 + p1 + const_row[None, :]
    if _trace:
        kernel._last_exec_ns = res.exec_time_ns
        kernel._last_trace = res.instructions_and_trace
    return outp


# revision 14
# speedup vs baseline: 1.0063x; 1.0063x over previous
"""Cross-attention kernel for TRN2, 8 NeuronCores.

Sharding: core c -> (batch b = c//2, head-group g = c%2).  Each head-group is
8 heads = 512 of the 1024 d_model channels.  Within a core everything runs in
one fused software pipeline over 4 head-pair stages (o = 0..3):

  QT = wq_g.T @ q.T  (scale folded)       [512, 512]   (s, lq)
  KT = wk_g.T @ kv.T                      [512, 2048]  (s, lkv)
  V  = kv @ wv_g.T                        [2048, 512]  (lkv, s)   + ones col
  phase1(o), t = 0..15:
               ST = Kh.T-contract @ QT    [128, 512]   (lkv-tile, lq)
               P[t] = exp(ST)          -> bf16 SBUF [128, 16, 512] per head
  phase2(o), unit (hp, lt):  16 consecutive matmuls in ONE psum bank
               ctx[lq, 65] += P[t]_lt.T @ [Vh | 1]     (F=65 transposed form;
                                                        col 64 = softmax denom)
               C = ctx[:, 0:64] * recip(ctx[:, 64])  (DVE per-partition scalar)
  transpose C -> cT[s, lq]  (PE transpose, identity trick)
  out = cT.T @ wo_g.T                     [512, 1024]
Host sums the two head-group partials per batch and adds bo + bv@Wo.T
(the V bias commutes through softmax-weighted averaging: sum(a)=1, so
ctx = sum(a v) + bv and the bv term is a constant row added on host; the
K bias drops entirely -- a per-row constant shift of the scores cancels
in softmax).

All operands are bf16 (1 cyc/row on PE at any free size; halves DMA), psum
accumulation f32.  phase2(o-1) and the projection matmuls of stage o+1 are
hand-interleaved into phase1(o)'s t-loop so the Act engine's exp stream (the
second largest engine load) fully overlaps PE work.  A psum accumulation
group owns its whole 2KB bank (start zeroes the full zero-region), hence the
consecutive-16 structure of phase2 rather than round-robin accumulation.

Timing model notes (TimelineSim is the metric): the PE clock ramp anchors at
the FIRST matmul and never resets on the gap sizes this kernel produces, so
three dummy matmuls right after the preamble are enough to have the ramp done
(3us) before the first real projection; DMA transfers serialize on the single
DMA_ENGINES device, so the input DMA order below is the exact just-in-time
consumption order of the stage-0 pipeline; the kernel tail is one
copy+DMA chain, minimized by making the final out-projection accumulation
group a [128,128] column slice whose DMA is the only one in flight.
"""

import sys
if "/opt/trn_rl_repo" not in sys.path:
    sys.path.insert(0, "/opt/trn_rl_repo")

import numpy as np
import ml_dtypes

import concourse.bass as bass
import concourse.mybir as mybir
import concourse.tile as tile
from concourse.bass_utils import run_bass_kernel_spmd

f32 = mybir.dt.float32
bf16 = mybir.dt.bfloat16
EXP = mybir.ActivationFunctionType.Exp
IDENT = mybir.ActivationFunctionType.Identity

D = 1024        # d_model
S = 512         # per-core channel shard (8 heads x 64)
LQ = 512
LKV = 2048
CO = D // 128   # 8 contraction chunks
SO = S // 128   # 4 shard s-tiles (head pairs)
NT = LKV // 128  # 16 lkv tiles


def _split_multi_waits(nc, max_waits=1):
    """This container's walrus allows only `max_waits` sync-wait commands per
    instruction; hoist the excess into standalone EventSemaphore insts."""
    ev_id = 0
    for f in nc.m.functions:
        for bb in f.blocks:
            new = []
            changed = False
            for inst in bb.instructions:
                si = inst.sync_info
                if si is not None and si.on_wait and len(si.on_wait) > max_waits:
                    waits = list(si.on_wait)
                    for sw in waits[:-max_waits]:
                        ev = mybir.InstEventSemaphore(
                            name=f"EVSPLIT-{ev_id}", engine=inst.engine,
                            sync_info=mybir.SyncInfo(on_wait=[sw], on_update=[]))
                        ev_id += 1
                        nc.register_instruction(ev, overwrite=True)
                        new.append(ev)
                    inst.sync_info = mybir.SyncInfo(
                        on_wait=waits[-max_waits:], on_update=list(si.on_update))
                    changed = True
                new.append(inst)
            if changed:
                bb.instructions = new
    return nc


def _build():
    nc = bass.Bass(trn_type="TRN2")

    # DRAM I/O (pre-laid-out [128, outer, free] on host, bf16)
    qT = nc.dram_tensor("qT", [128, CO, LQ], bf16, kind="ExternalInput")
    kvT = nc.dram_tensor("kvT", [128, CO, LKV], bf16, kind="ExternalInput")
    # o=0 slices of Wq|Wk packed contiguously: one full-rate DMA on the
    # critical path to the first score matmul (256-col slices of wqT/wkT
    # alone pay the sub-512B DMA penalty)
    wqk0 = nc.dram_tensor("wqk0", [128, CO, 256], bf16, kind="ExternalInput")
    # o=1..3 slices of Wq|Wk packed: cols 0:384 = wq[:, :, 128:512],
    # cols 384:768 = wk[:, :, 128:512]
    wqkr = nc.dram_tensor("wqkr", [128, CO, 768], bf16, kind="ExternalInput")
    wvT = nc.dram_tensor("wvT", [128, CO, S], bf16, kind="ExternalInput")
    woT = nc.dram_tensor("woT", [128, SO, D], bf16, kind="ExternalInput")
    bq = nc.dram_tensor("bq", [128, SO], f32, kind="ExternalInput")
    ident = nc.dram_tensor("ident", [128, 128], bf16, kind="ExternalInput")
    out = nc.dram_tensor("out", [SO, 128, D], bf16, kind="ExternalOutput")

    with tile.TileContext(nc) as tc:
        with tc.tile_pool(name="wgt", bufs=1) as wgt, \
             tc.tile_pool(name="pt", bufs=2) as ptp, \
             tc.tile_pool(name="stg", bufs=4) as stg, \
             tc.tile_pool(name="ost", bufs=3) as ost, \
             tc.tile_pool(name="ps", bufs=1, space="PSUM") as ps:

            # ---- resident SBUF ----
            kv_sb = wgt.tile([128, CO, LKV], bf16, name="kv_sb")
            wqkr_sb = wgt.tile([128, CO, 768], bf16, name="wqkr_sb")
            wv_sb = wgt.tile([128, CO, S], bf16, name="wv_sb")
            wo_sb = wgt.tile([128, SO, D], bf16, name="wo_sb")
            qT_sb = wgt.tile([128, CO, LQ], bf16, name="qT_sb")
            QT_sb = wgt.tile([128, SO, LQ], bf16, name="QT_sb")
            KT_sb = wgt.tile([128, SO, LKV], bf16, name="KT_sb")
            # V per head with a ones column: attn@V (transposed form) then
            # also yields the softmax denominator in output col 64.
            Vp_sb = wgt.tile([128, NT, 8, 65], bf16, name="Vp_sb")
            cT_sb = wgt.tile([128, SO, LQ], bf16, name="cT_sb")
            bq_sb = wgt.tile([128, SO], f32, name="bq_sb")
            ident_sb = wgt.tile([128, 128], bf16, name="ident_sb")
            wqk0_sb = wgt.tile([128, CO, 256], bf16, name="wqk0_sb")

            # ---- PE clock-ramp anchor: the ramp timer starts at the first
            # matmul and survives the idle gaps this kernel produces, so a
            # few cheap dummies right after the preamble put the engine at
            # full clock (3us later) before the first real projection.
            dm_sb = wgt.tile([128, 512], bf16, name="dm_sb")
            nc.vector.memset(dm_sb, 0.0)
            for i in range(8):
                dps = ps.tile([128, 512], f32, name=f"dps{i}", tag="proj",
                              bufs=2)
                nc.tensor.matmul(dps, dm_sb[:, 0:128], dm_sb,
                                 start=True, stop=True)

            # ---- DMA order = just-in-time consumption order (transfers
            # serialize on the DMA_ENGINES device).  Stage 0 needs the o=0
            # weight slices, q, and the kv chunks in score order; weights for
            # later stages and the out-projection arrive behind them.
            nc.sync.dma_start(wqk0_sb, wqk0[:])
            nc.sync.dma_start(qT_sb[:, 0:4, :], qT[:, 0:4, :])
            nc.sync.dma_start(qT_sb[:, 4:8, :], qT[:, 4:8, :])
            nc.sync.dma_start(kv_sb[:, 0:4, 0:512], kvT[:, 0:4, 0:512])
            nc.sync.dma_start(kv_sb[:, 4:8, 0:512], kvT[:, 4:8, 0:512])
            nc.sync.dma_start(bq_sb, bq[:])
            for hh in range(2, 8):
                nc.sync.dma_start(kv_sb[:, 4 * (hh % 2):4 * (hh % 2) + 4,
                                        (hh // 2) * 512:(hh // 2 + 1) * 512],
                                  kvT[:, 4 * (hh % 2):4 * (hh % 2) + 4,
                                      (hh // 2) * 512:(hh // 2 + 1) * 512])
            nc.sync.dma_start(wv_sb[:, :, 0:256], wvT[:, :, 0:256])
            nc.sync.dma_start(wv_sb[:, :, 256:512], wvT[:, :, 256:512])
            nc.sync.dma_start(ident_sb, ident[:])
            nc.sync.dma_start(wqkr_sb, wqkr[:])
            nc.sync.dma_start(wo_sb, woT[:])

            nc.vector.memset(Vp_sb[:, :, :, 64:65], 1.0)

            # ---- emission helpers (each emits PE matmuls + its drain) ----
            kps_open = {}   # (o, ch) -> open psum accumulation tile

            def kproj_half(o, ch, half):
                """Half of a K-projection accumulation group: c 0..3 (opens
                the psum group) or c 4..7 (closes it + drains to KT).  Stage
                0's kv chunks land one per ~1.5us, so splitting emission at
                the chunk boundary keeps the in-order PE from stalling a
                whole group's worth of score tiles behind one late chunk.
                NOTE: no other proj-tag tile may be allocated while a group
                is open (the 2-buffer rotation would land on the open bank).
                """
                if half == 0:
                    kps_open[(o, ch)] = ps.tile(
                        [128, 512], f32, name=f"kps{o}_{ch}", tag="proj",
                        bufs=2)
                kps = kps_open[(o, ch)]
                sl = slice(ch * 512, (ch + 1) * 512)
                for c in range(4 * half, 4 * half + 4):
                    w = (wqk0_sb[:, c, 128:256] if o == 0 else
                         wqkr_sb[:, c, 384 + (o - 1) * 128:384 + o * 128])
                    nc.tensor.matmul(kps, w, kv_sb[:, c, sl],
                                     start=(c == 0), stop=(c == CO - 1))
                if half == 1:
                    del kps_open[(o, ch)]
                    nc.vector.tensor_copy(KT_sb[:, o, sl], kps)

            def kproj(o, ch):
                kproj_half(o, ch, 0)
                kproj_half(o, ch, 1)

            def qproj(o):
                qps = ps.tile([128, 512], f32, name=f"qps{o}", tag="proj",
                              bufs=2)
                for c in range(CO):
                    w = (wqk0_sb[:, c, 0:128] if o == 0 else
                         wqkr_sb[:, c, (o - 1) * 128:o * 128])
                    nc.tensor.matmul(qps, w, qT_sb[:, c, :],
                                     start=(c == 0), stop=(c == CO - 1))
                nc.vector.tensor_scalar_add(QT_sb[:, o, :], qps,
                                            bq_sb[:, o:o + 1])

            def vproj(o, t):
                vps = ps.tile([128, 128], f32, name=f"vps{o}_{t}", tag="proj",
                              bufs=2)
                tsl = slice(t * 128, (t + 1) * 128)
                osl = slice(o * 128, (o + 1) * 128)
                for c in range(CO):
                    nc.tensor.matmul(vps, kv_sb[:, c, tsl], wv_sb[:, c, osl],
                                     start=(c == 0), stop=(c == CO - 1))
                nc.vector.tensor_copy(
                    Vp_sb[:, t, 2 * o:2 * o + 2, 0:64],
                    vps.rearrange("p (h d) -> p h d", h=2))

            # ---- lead-in: stage-0 prerequisites ----
            qproj(0)
            kproj(0, 0)

            # Per-stage fill schedules: iteration t -> thunks.  Placement
            # matches DMA arrival order (PE is in-order, so emitting a matmul
            # whose DMA lands late would stall everything behind it).
            def mk_sched(o):
                s = {t: [] for t in range(NT)}
                if o == 0:
                    # kv chunks land one per ~1.5us; kproj(0,ch) feeds the
                    # scores at t=4ch.  Emit each kproj as two chunk-halves
                    # at consecutive t so the PE stall per late chunk is
                    # spread across score tiles.  wv lands after kv7, so the
                    # V projections start one tile later than the kproj
                    # halves they share proj-tag banks with.
                    s[0].append(lambda: kproj_half(0, 1, 0))
                    s[1].append(lambda: kproj_half(0, 1, 1))
                    s[4].append(lambda: kproj_half(0, 2, 0))
                    s[5].append(lambda: kproj_half(0, 2, 1))
                    s[8].append(lambda: kproj_half(0, 3, 0))
                    s[9].append(lambda: kproj_half(0, 3, 1))
                    nv = 0
                    for t in range(9, NT):
                        take = 2 if t < 14 else 3
                        for _ in range(take):
                            if nv < NT:
                                s[t].append(lambda v=nv: vproj(0, v))
                                nv += 1
                else:
                    # own K chunks 1..3 first (ch0/qproj ran at the tail of
                    # the previous stage), V tiles just-in-time for phase2.
                    for ch in range(1, 4):
                        s[ch - 1].append(lambda ch=ch: kproj(o, ch))
                    for t in range(NT):
                        s[t].append(lambda t=t: vproj(o, t))
                if o < 3:
                    # next stage's first K chunk before its Q projection:
                    # the next stage's first score tile is gated by the
                    # KT-copy drain, so give it the extra headroom
                    s[NT - 2].append(lambda: kproj(o + 1, 0))
                    s[NT - 1].append(lambda: qproj(o + 1))
                return s

            def phase2_unit(o, pt, hp, lt, c_sb):
                """ctx unit (head hp of pair o, lq tile lt): 16 consecutive
                matmuls in one psum bank, then normalize straight from psum.
                (An accumulation group owns its whole 2KB zero-region, so the
                16 steps must be consecutive in one dedicated bank.)
                Pair 3 runs at the kernel tail where Act is idle, so its
                normalize goes to the scalar engine instead of DVE, and its
                units alternate over the then-idle proj banks as well to keep
                4 accumulations in flight instead of 2."""
                if o == SO - 1:
                    # score banks are idle after stage 3's last exp: rotate
                    # over ctx/st (4 buffers) so accumulations stay in
                    # flight; proj stays exclusive to the transposes
                    tag = ("ctx", "st")[(hp * SO + lt) % 2]
                else:
                    tag = "ctx"
                ctx = ps.tile([128, 65], f32, name=f"ctx{o}_{hp}_{lt}",
                              tag=tag, bufs=2)
                base = hp * 512 + lt * 128
                for t in range(NT):
                    nc.tensor.matmul(
                        ctx, pt[:, t, base:base + 128],
                        Vp_sb[:, t, 2 * o + hp, :],
                        start=(t == 0), stop=(t == NT - 1))
                rc = stg.tile([128, 1], f32, name=f"rc{o}_{hp}_{lt}", tag="rc",
                              bufs=4)
                nc.vector.reciprocal(rc, ctx[:, 64:65])
                if o == SO - 1:
                    nc.scalar.activation(c_sb[:, hp, lt, :], ctx[:, 0:64],
                                         IDENT, scale=rc)
                else:
                    nc.vector.tensor_scalar_mul(
                        c_sb[:, hp, lt, :], ctx[:, 0:64], rc)

            def transpose_pair(o, hp, c_sb):
                trp = ps.tile([128, SO, 128], bf16, name=f"trp{o}_{hp}",
                              tag="proj", bufs=2)
                for lt in range(SO):
                    nc.tensor.transpose(trp[0:64, lt, :],
                                        c_sb[:, hp, lt, :], ident_sb)
                nc.vector.tensor_copy(
                    cT_sb[hp * 64:(hp + 1) * 64, o, :],
                    trp[0:64, :, :].rearrange("p a b -> p (a b)"))

            def phase2_steps(o, pt):
                """Thunks: 8 ctx units + 2 transposes for pair-stage o."""
                c_sb = stg.tile([128, 2, SO, 64], bf16, name=f"c{o}", tag="c",
                                bufs=2)
                for hp in range(2):
                    for lt in range(SO):
                        yield lambda hp=hp, lt=lt: phase2_unit(
                            o, pt, hp, lt, c_sb)
                    yield lambda hp=hp: transpose_pair(o, hp, c_sb)

            # ---- 4 head-pair stages ----
            prev_p2 = None   # phase2 step iterator of the previous stage
            for o in range(SO):
                sched = mk_sched(o)
                pt = ptp.tile([128, NT, 1024], bf16, name=f"pt{o}",
                              tag="pt", bufs=2)
                for t in range(NT):
                    # fused score tile: head 2o in bank cols 0:512, head
                    # 2o+1 in 512:1024 (each matmul stays within one bank)
                    st2 = ps.tile([128, 1024], f32, name=f"st{o}_{t}",
                                  tag="st", bufs=2)
                    tsl = slice(t * 128, (t + 1) * 128)
                    nc.tensor.matmul(st2[:, 0:512], KT_sb[0:64, o, tsl],
                                     QT_sb[0:64, o, :], start=True, stop=True)
                    nc.tensor.matmul(st2[:, 512:1024], KT_sb[64:128, o, tsl],
                                     QT_sb[64:128, o, :], start=True, stop=True)
                    nc.scalar.activation(pt[:, t, :], st2, EXP)
                    # one phase2 step of the previous stage every other t
                    if t % 2 == 1 and prev_p2 is not None:
                        step = next(prev_p2, None)
                        if step is not None:
                            step()
                        if t == NT - 1:  # 10 steps total, drain leftovers
                            for step in prev_p2:
                                step()
                    for thunk in sched[t]:
                        thunk()
                prev_p2 = phase2_steps(o, pt)
            for step in prev_p2:
                step()

            # ---- out projection: out[lq, d] += cT[:, o, lq-sl].T @ wo ----
            # The kernel end is gated by the LAST out-DMA chain: sem(~200) +
            # copy + HWDGE(625, single slot) + DGE(650) + transfer + 900.
            # Any other DMA issued within ~1.3us before it queues ahead of
            # it on HWDGE and becomes the laggard instead.  So the final lq
            # tile's last 128 columns are produced by TWO tiny [64]-column
            # groups (no DMA of their own) merged into ONE short final DMA,
            # and every other group's DMA is issued >= 2 group-slots
            # (~1.8us) before the end.
            ogroups = [
                (3, 0, 512, "act", "piece"),
                (0, 0, 512, "act", None), (0, 512, 1024, "dve", "full"),
                (1, 0, 512, "act", None), (1, 512, 1024, "dve", "full"),
                (2, 0, 512, "act", None), (2, 512, 1024, "dve", "full"),
            ] + [
                # lt3's high half as four DMA-less [128]-col groups: ~0.9us
                # of PE work between the last big out-DMA (lt2) and kernel
                # end, so only the single merged DMA below is in flight at
                # the finish line.
                (3, 512 + 128 * i, 640 + 128 * i, ("act", "dve")[i % 2],
                 "tail" if i == 3 else None)
                for i in range(4)
            ]
            ot_tiles = {}
            for lt in range(SO):
                ot_tiles[lt] = ost.tile([128, D], bf16, name=f"ot{lt}",
                                        tag=f"ot{lt}")
            # big groups alternate proj/st; the four tail groups spread over
            # three tags so the 107-214ns groups never wait a buffer whose
            # copy is still draining
            otags = ["proj", "st", "proj", "st", "proj", "st", "proj",
                     "st", "ctx", "proj", "ctx"]
            for gi, (lt, c0, c1, eng, dma) in enumerate(ogroups):
                lsl = slice(lt * 128, (lt + 1) * 128)
                ops = ps.tile([128, c1 - c0], f32, name=f"ops{gi}",
                              tag=otags[gi], bufs=2)
                for o in range(SO):
                    nc.tensor.matmul(ops, cT_sb[:, o, lsl],
                                     wo_sb[:, o, c0:c1],
                                     start=(o == 0), stop=(o == SO - 1))
                ot = ot_tiles[lt]
                if eng == "act":
                    nc.scalar.activation(ot[:, c0:c1], ops, IDENT)
                else:
                    nc.vector.tensor_copy(ot[:, c0:c1], ops)
                if dma == "piece":
                    nc.sync.dma_start(out[lt, :, c0:c1], ot[:, c0:c1])
                elif dma == "full":
                    nc.sync.dma_start(out[lt, :, :], ot)
                elif dma == "tail":
                    # covers all eight [64]-col groups; waits all copies
                    nc.sync.dma_start(out[lt, :, 512:1024], ot[:, 512:1024])

    return _split_multi_waits(nc)


_NC = None


def _get_nc():
    global _NC
    if _NC is None:
        _NC = _build()
    return _NC


def _shard(q, kv, Wq, bq, Wk, bk, Wv, bv, Wo, bo):
    b16 = ml_dtypes.bfloat16

    def lay(a2d, co):  # [co*128, F] -> [128, co, F]
        F = a2d.shape[1]
        return np.ascontiguousarray(
            a2d.reshape(co, 128, F).transpose(1, 0, 2)).astype(b16)

    idn = np.eye(128, dtype=b16)
    in_maps = []
    for core in range(8):
        b, g = core // 2, core % 2
        sl = slice(g * S, (g + 1) * S)
        wq_l = lay(np.ascontiguousarray((Wq[sl] * 0.125).T), CO)
        wk_l = lay(np.ascontiguousarray(Wk[sl].T), CO)
        m = {
            "wqk0": np.ascontiguousarray(
                np.concatenate([wq_l[:, :, 0:128], wk_l[:, :, 0:128]],
                               axis=2)),
            "wqkr": np.ascontiguousarray(
                np.concatenate([wq_l[:, :, 128:512], wk_l[:, :, 128:512]],
                               axis=2)),
            "qT": lay(np.ascontiguousarray(q[b].T), CO),
            "kvT": lay(np.ascontiguousarray(kv[b].T), CO),
            "wvT": lay(np.ascontiguousarray(Wv[sl].T), CO),
            "woT": lay(np.ascontiguousarray(Wo[:, sl].T), SO),
            "bq": np.ascontiguousarray(
                (bq[sl] * 0.125).reshape(SO, 128).T).astype(np.float32),
            "ident": idn,
        }
        in_maps.append(m)
    return in_maps


def _run(in_maps, trace=False):
    res = run_bass_kernel_spmd(_get_nc(), in_maps, core_ids=list(range(8)),
                               trace=trace)
    return res


def kernel(q, kv, Wq, bq, Wk, bk, Wv, bv, Wo, bo, _trace=False):
    q, kv = np.asarray(q, np.float32), np.asarray(kv, np.float32)
    Wq, Wk = np.asarray(Wq, np.float32), np.asarray(Wk, np.float32)
    Wv, Wo = np.asarray(Wv, np.float32), np.asarray(Wo, np.float32)
    bq, bk = np.asarray(bq, np.float32), np.asarray(bk, np.float32)
    bv, bo = np.asarray(bv, np.float32), np.asarray(bo, np.float32)

    in_maps = _shard(q, kv, Wq, bq, Wk, bk, Wv, bv, Wo, bo)
    res = _run(in_maps, trace=_trace)
    B = q.shape[0]
    # bv commutes through the softmax average; bk cancels in softmax.
    const_row = bv @ Wo.T + bo
    outp = np.empty((B, LQ, D), np.float32)
    for b in range(B):
        p0 = np.asarray(res.results[2 * b]["out"],
                        np.float32).reshape(LQ, D)
        p1 = np.asarray(res.results[2 * b + 1]["out"],
                        np.float32).reshape(LQ, D)
        outp[b] = p0 + p1 + const_row[None, :]
    if _trace:
        kernel._last_exec_ns = res.exec_time_ns
        kernel._last_trace = res.instructions_and_trace
    return outp
